# revision 1
# baseline (speedup 1.0000x reference)
"""SSIM loss kernel for Trainium2 (Bass/Tile), 8-core data parallel.

Math (per 512x512 plane, 11x11 gaussian window G, zero "same" padding):
  mu_x = G*X, mu_y = G*Y
  ssim = ((2 mu_x mu_y + C1)(2 sg_xy + C2)) / ((mu_x^2+mu_y^2+C1)(sg_x+sg_y+C2))
  loss = 1 - mean(ssim)

Reformulation used here (s/d trick):
  F1 = X+Y, F2 = X-Y, uF = F1^2/2, vF = F2^2/2
  s = G*F1, d = G*F2   (blur is linear)
  u = s^2/2, v = d^2/2
  bU = G*uF + G*vF = G*(X^2+Y^2),  bW = G*uF - G*vF = 2 G*(XY)
  A1 = u - v + C1          (= 2 mu_x mu_y + C1)
  B1 = u + v + C1          (= mu_x^2 + mu_y^2 + C1)
  A2 = bW + (C1+C2) - A1   (= 2 sg_xy + C2)
  B2 = bU + (C1+C2) - B1   (= sg_x + sg_y + C2)
  ssim = (A1*A2) / (B1*B2)

The separable blur runs on the TensorEngine as banded matmuls with the image
block as the stationary operand: matmul(out, lhsT=img_block, rhs=bandmat)
computes (A @ img)^T, i.e. a 1-D conv along the partition axis plus a free
transpose.  Two such passes give the 2-D blur and return to natural layout.

Each core handles 2 of the 16 batch images (6 planes), produces per-partition
partial sums [128, 24]; the host sums everything in float64.
"""

import sys

for _p in ("/opt/trn_rl_repo",):
    if _p not in sys.path:
        sys.path.insert(0, _p)

import numpy as np

import concourse.bass as bass
import concourse.bacc as bacc
import concourse.mybir as mybir
import concourse.tile as tile
from concourse.bass_utils import run_bass_kernel_spmd

F32 = mybir.dt.float32
AOP = mybir.AluOpType
AFT = mybir.ActivationFunctionType

N_CORES = 8
BATCH = 16
CH = 3
H = W = 512
PLANES = (BATCH // N_CORES) * CH  # 6 planes per core
WIN_SIZE = 11
SIGMA = 1.5
HALF = WIN_SIZE // 2
C1 = 0.01 ** 2
C2 = 0.03 ** 2
CC = C1 + C2
INVR2 = float(np.float32(1.0) / np.sqrt(np.float32(2.0)))

# per k-tile output-row windows [nstart, nstart+width) and offsets into the
# concatenated band matrix
WIN = [(0, 133), (123, 138), (251, 138), (379, 133)]
OFF = [0, 133, 271, 409]
CATW = 542  # 133+138+138+133

# aligned mode: dense per-k-tile segments of width 512; matmul windows are
# 64-element-aligned sub-slices (first-source splits at the written boundary)
AWIN_FIRST = [  # per kt: list of (nstart, width, accumulate_onto_written)
    [(0, 256)],
    [(64, 192), (256, 64)],
    [(192, 128), (320, 128)],
    [(320, 128), (448, 64)],
]
AWIN_SECOND = [(0, 256), (64, 256), (192, 256), (320, 192)]
DCATW = 2048


def _gauss1d():
    coords = np.arange(WIN_SIZE, dtype=np.float32) - HALF
    g = np.exp(-(coords ** 2) / np.float32(2.0 * SIGMA ** 2)).astype(np.float32)
    g = g / g.sum(dtype=np.float32)
    return g.astype(np.float32)


def _band_matrix_np():
    """[128, 2*542]: concat of the 4 per-k-tile band segments, then negated."""
    g = _gauss1d()
    A = np.zeros((H, H), dtype=np.float32)
    for i in range(H):
        lo = max(0, i - HALF)
        hi = min(H, i + HALF + 1)
        for j in range(lo, hi):
            A[i, j] = g[j - i + HALF]
    segs = []
    for kt in range(4):
        ns, w = WIN[kt]
        # R_kt[k', n] = A[n, kt*128+k']  -> shape [128, w]
        segs.append(A[ns:ns + w, kt * 128:(kt + 1) * 128].T.copy())
    cat = np.concatenate(segs, axis=1)
    assert cat.shape == (128, CATW)
    return np.concatenate([cat, -cat], axis=1).astype(np.float32)


def _band_matrix_dense_np():
    """[128, 2*2048]: dense 512-wide per-k-tile segments, then negated."""
    g = _gauss1d()
    A = np.zeros((H, H), dtype=np.float32)
    for i in range(H):
        lo = max(0, i - HALF)
        hi = min(H, i + HALF + 1)
        for j in range(lo, hi):
            A[i, j] = g[j - i + HALF]
    segs = []
    for kt in range(4):
        segs.append(A[:, kt * 128:(kt + 1) * 128].T.copy())  # [128, 512]
    cat = np.concatenate(segs, axis=1)
    assert cat.shape == (128, DCATW)
    return np.concatenate([cat, -cat], axis=1).astype(np.float32)


def build_nc(planes=PLANES, dma="gpsimd", recip="stock", act_bias="auto",
             stop_after="full", conv="aligned", post="vanilla"):
    nc = bacc.Bacc(None)
    dmae = {"gpsimd": nc.gpsimd, "sync": nc.sync}[dma]
    pred_d = nc.declare_dram_parameter("pred", [planes, H, W], F32, isOutput=False)
    targ_d = nc.declare_dram_parameter("target", [planes, H, W], F32, isOutput=False)
    bandw = 2 * (CATW if conv == "banded" else DCATW)
    band_d = nc.declare_dram_parameter("bandmat", [128, bandw], F32, isOutput=False)
    acc_d = nc.declare_dram_parameter("acc", [128, planes * 4], F32, isOutput=True)
    dbg_d = (nc.declare_dram_parameter("dbg", [128, 2048], F32, isOutput=True)
             if stop_after == "pass1" else None)

    with tile.TileContext(nc) as tc:
        with (
            tc.tile_pool(name="const", bufs=1) as constp,
            tc.tile_pool(name="xy", bufs=2) as xyp,
            tc.tile_pool(name="fields", bufs=1) as fldp,
            tc.tile_pool(name="transposed", bufs=1) as trp,
            tc.tile_pool(name="post", bufs=2) as pp,
            tc.tile_pool(name="accp", bufs=1) as accp,
            tc.tile_pool(name="ps1", bufs=4, space="PSUM") as ps1,
            tc.tile_pool(name="ps2", bufs=1, space="PSUM") as ps2,
        ):
            BM = constp.tile([128, bandw], F32)
            dmae.dma_start(BM[:], band_d[:])
            acc = accp.tile([128, planes * 4], F32)
            if act_bias == "tile":
                zb = constp.tile([128, 1], F32)
                nc.vector.memset(zb[:], 0.0)
                bias_kw = {"bias": zb[:]}
            else:
                bias_kw = {}

            def conv_pass(dst_psum, srcs, blk):
                """dst_psum[p, n] += sum over srcs of 1-D conv along partition
                axis of src block `blk` (128-col block), transposed."""
                mms = []
                for si, (T, neg) in enumerate(srcs):
                    for kt in range(4):
                        lhsT = T[:, kt * 512 + blk * 128: kt * 512 + (blk + 1) * 128]
                        if conv == "banded":
                            ns, w = WIN[kt]
                            off = OFF[kt] + (CATW if neg else 0)
                            if si == 0 and kt > 0:
                                mms.append((dst_psum[:, ns:ns + 10],
                                            lhsT, BM[:, off:off + 10]))
                                mms.append((dst_psum[:, ns + 10:ns + w],
                                            lhsT, BM[:, off + 10:off + w]))
                            else:
                                mms.append((dst_psum[:, ns:ns + w],
                                            lhsT, BM[:, off:off + w]))
                        else:
                            off = kt * 512 + (DCATW if neg else 0)
                            if si == 0:
                                wins = AWIN_FIRST[kt]
                            else:
                                wins = [AWIN_SECOND[kt]]
                            for ns, w in wins:
                                mms.append((dst_psum[:, ns:ns + w],
                                            lhsT, BM[:, off + ns:off + ns + w]))
                n_mm = len(mms)
                for i, (o, l, r) in enumerate(mms):
                    nc.tensor.matmul(o, l, r, start=(i == 0), stop=(i == n_mm - 1))

            for p in range(planes):
                X = xyp.tile([128, 2048], F32, tag="X")
                Y = xyp.tile([128, 2048], F32, tag="Y")
                dmae.dma_start(
                    X[:].rearrange("q (kt c) -> q kt c", kt=4),
                    pred_d[p].rearrange("(kt q) c -> q kt c", q=128))
                dmae.dma_start(
                    Y[:].rearrange("q (kt c) -> q kt c", kt=4),
                    targ_d[p].rearrange("(kt q) c -> q kt c", q=128))

                F1 = fldp.tile([128, 2048], F32, tag="F1")
                F2 = fldp.tile([128, 2048], F32, tag="F2")
                uF = fldp.tile([128, 2048], F32, tag="uF")
                vF = fldp.tile([128, 2048], F32, tag="vF")
                nc.vector.tensor_tensor(F1[:], X[:], Y[:], AOP.add)
                nc.vector.tensor_tensor(F2[:], X[:], Y[:], AOP.subtract)
                nc.scalar.activation(uF[:], F1[:], AFT.Square, scale=INVR2, **bias_kw)
                nc.scalar.activation(vF[:], F2[:], AFT.Square, scale=INVR2, **bias_kw)

                Ts = {}
                for nmf, ft in (("F1", F1), ("F2", F2), ("uF", uF), ("vF", vF)):
                    T = trp.tile([128, 2048], F32, tag="T" + nmf)
                    for mc in range(4):
                        ps = ps1.tile([128, 512], F32, tag="p1")
                        conv_pass(ps, [(ft, False)], mc)
                        nc.scalar.copy(T[:, mc * 512:(mc + 1) * 512], ps[:])
                    Ts[nmf] = T

                if stop_after == "pass1":
                    dmae.dma_start(dbg_d[:], Ts["uF"][:])
                    continue

                for rc in range(4):
                    pss = ps2.tile([128, 512], F32, tag="pss")
                    psd = ps2.tile([128, 512], F32, tag="psd")
                    psU = ps2.tile([128, 512], F32, tag="psU")
                    psW = ps2.tile([128, 512], F32, tag="psW")
                    conv_pass(pss, [(Ts["F1"], False)], rc)
                    conv_pass(psd, [(Ts["F2"], False)], rc)
                    conv_pass(psU, [(Ts["uF"], False), (Ts["vF"], False)], rc)
                    conv_pass(psW, [(Ts["uF"], False), (Ts["vF"], True)], rc)

                    u = pp.tile([128, 512], F32, tag="u")
                    v = pp.tile([128, 512], F32, tag="v")
                    nc.scalar.activation(u[:], pss[:], AFT.Square, scale=INVR2, **bias_kw)
                    nc.scalar.activation(v[:], psd[:], AFT.Square, scale=INVR2, **bias_kw)

                    A1 = pp.tile([128, 512], F32, tag="A1")
                    B1 = pp.tile([128, 512], F32, tag="B1")
                    A2 = pp.tile([128, 512], F32, tag="A2")
                    B2 = pp.tile([128, 512], F32, tag="B2")
                    Nt = pp.tile([128, 512], F32, tag="Nt")
                    Dt = pp.tile([128, 512], F32, tag="Dt")
                    Rt = pp.tile([128, 512], F32, tag="Rt")
                    ssim = pp.tile([128, 512], F32, tag="ssim")
                    if post == "fused":
                        nc.vector.scalar_tensor_tensor(A1[:], u[:], C1, v[:], AOP.add, AOP.subtract)
                        nc.vector.scalar_tensor_tensor(B1[:], u[:], C1, v[:], AOP.add, AOP.add)
                        nc.vector.scalar_tensor_tensor(A2[:], psW[:], CC, A1[:], AOP.add, AOP.subtract)
                        nc.vector.scalar_tensor_tensor(B2[:], psU[:], CC, B1[:], AOP.add, AOP.subtract)
                        nc.vector.tensor_tensor(Nt[:], A1[:], A2[:], AOP.mult)
                        nc.vector.tensor_tensor(Dt[:], B1[:], B2[:], AOP.mult)
                        scr = pp.tile([128, 512], F32, tag="scr")
                        if recip == "approx":
                            nc.vector.reciprocal_approx_accurate(Rt[:], Dt[:], scr[:])
                        else:
                            nc.vector.reciprocal(Rt[:], Dt[:])
                        nc.vector.tensor_tensor_reduce(
                            out=ssim[:], in0=Nt[:], in1=Rt[:], scale=1.0,
                            scalar=0.0, op0=AOP.mult, op1=AOP.add,
                            accum_out=acc[:, p * 4 + rc: p * 4 + rc + 1])
                    else:
                        t1 = pp.tile([128, 512], F32, tag="t1")
                        t2 = pp.tile([128, 512], F32, tag="t2")
                        t3 = pp.tile([128, 512], F32, tag="t3")
                        wks = pp.tile([128, 512], F32, tag="wks")
                        uks = pp.tile([128, 512], F32, tag="uks")
                        nc.vector.tensor_scalar_add(t1[:], u[:], C1)
                        nc.vector.tensor_tensor(A1[:], t1[:], v[:], AOP.subtract)
                        nc.vector.tensor_tensor(B1[:], t1[:], v[:], AOP.add)
                        nc.vector.tensor_scalar_add(wks[:], psW[:], CC)
                        nc.vector.tensor_scalar_add(uks[:], psU[:], CC)
                        nc.vector.tensor_tensor(A2[:], wks[:], A1[:], AOP.subtract)
                        nc.vector.tensor_tensor(B2[:], uks[:], B1[:], AOP.subtract)
                        nc.vector.tensor_tensor(Nt[:], A1[:], A2[:], AOP.mult)
                        nc.vector.tensor_tensor(Dt[:], B1[:], B2[:], AOP.mult)
                        nc.vector.reciprocal(Rt[:], Dt[:])
                        nc.vector.tensor_tensor(ssim[:], Nt[:], Rt[:], AOP.mult)
                        nc.vector.tensor_reduce(
                            acc[:, p * 4 + rc: p * 4 + rc + 1], ssim[:],
                            mybir.AxisListType.X, AOP.add)

            dmae.dma_start(acc_d[:], acc[:])
    nc.compile()
    return nc


_CACHE = {}


def _get_nc():
    if "nc" not in _CACHE:
        _CACHE["nc"] = build_nc()
        _CACHE["band"] = _band_matrix_dense_np()
    return _CACHE["nc"], _CACHE["band"]


def kernel(pred, target, _trace=False):
    pred = np.ascontiguousarray(np.asarray(pred), dtype=np.float32)
    target = np.ascontiguousarray(np.asarray(target), dtype=np.float32)
    nc, band = _get_nc()
    per = BATCH // N_CORES
    in_maps = []
    for i in range(N_CORES):
        in_maps.append({
            "pred": np.ascontiguousarray(
                pred[per * i: per * (i + 1)].reshape(PLANES, H, W)),
            "target": np.ascontiguousarray(
                target[per * i: per * (i + 1)].reshape(PLANES, H, W)),
            "bandmat": band,
        })
    kw = {}
    if _trace:
        kw["trace"] = True
    res = run_bass_kernel_spmd(nc, in_maps, list(range(N_CORES)), **kw)
    total = 0.0
    for r in res.results:
        total += float(np.asarray(r["acc"]).astype(np.float64).sum())
    loss = 1.0 - total / float(BATCH * CH * H * W)
    out = np.float32(loss)
    if _trace:
        return out, res
    return out



# revision 7
# speedup vs baseline: 2.0029x; 2.0029x over previous
"""SSIM loss kernel for Trainium2 (Bass/Tile), 8-core data parallel. v2: bf16.

Math (per 512x512 plane, 11x11 gaussian window G, zero "same" padding):
  mu_x = G*X, mu_y = G*Y
  ssim = ((2 mu_x mu_y + C1)(2 sg_xy + C2)) / ((mu_x^2+mu_y^2+C1)(sg_x+sg_y+C2))
  loss = 1 - mean(ssim)

Reformulation (s/d trick):
  F1 = X+Y, F2 = X-Y, uF = F1^2, vF = F2^2 (unscaled; the 1/2 is folded
  into the half-scaled band segment used for their pass-1 blur)
  s = G2(F1), d = G2(F2)        (G2 = 2-D blur, two banded-matmul passes)
  u = (s/sqrt2)^2, v = (d/sqrt2)^2
  psU = G2(uF)/2 + G2(vF)/2 = G2(X^2+Y^2)
  psW = G2(uF)/2 - G2(vF)/2 = 2 G2(XY)   (negated band for the vF stream)
  A1 = (u + C1) - v, B1 = (u + C1) + v
  A2 = (psW + CC) - A1, B2 = (psU + CC) - B1      (CC = C1+C2)
  ssim = (A1*A2) / (B1*B2)

All matmuls run in bf16 (1 col/cycle vs 4 for fp32). The blur is a banded
matmul with the image block stationary: matmul(out, lhsT=img_block,
rhs=band_cols) computes a 1-D conv along the partition axis plus a free
transpose; two passes give the separable 2-D blur back in natural layout.

Engine split per plane: GpSimd does F1/F2, ScalarE does squares + PSUM
extraction copies, VectorE does the bf16 algebra + reciprocal + reduce,
Sync triggers DMA. Host sums the per-partition partials in float64.
"""

import sys

for _p in ("/opt/trn_rl_repo",):
    if _p not in sys.path:
        sys.path.insert(0, _p)

import numpy as np
import ml_dtypes

import concourse.bass as bass
import concourse.bacc as bacc
import concourse.mybir as mybir
import concourse.tile as tile
from concourse.bass_utils import run_bass_kernel_spmd

F32 = mybir.dt.float32
BF16 = mybir.dt.bfloat16
AOP = mybir.AluOpType
AFT = mybir.ActivationFunctionType

N_CORES = 8
BATCH = 16
CH = 3
H = W = 512
PLANES = (BATCH // N_CORES) * CH  # 6 planes per core
WIN_SIZE = 11
SIGMA = 1.5
HALF = WIN_SIZE // 2
C1 = 0.01 ** 2
C2 = 0.03 ** 2
CC = C1 + C2
INVR2 = float(np.float32(1.0) / np.sqrt(np.float32(2.0)))

# per k-tile output-row windows [nstart, nstart+width) and offsets into one
# 542-wide band segment
WIN = [(0, 133), (123, 138), (251, 138), (379, 133)]
OFF = [0, 133, 271, 409]
CATW = 542  # 133+138+138+133
# band variants laid out side by side: positive, negated, half-scaled
VPOS, VNEG, VHALF = 0, 1, 2
BANDW = 3 * CATW


def _gauss1d():
    coords = np.arange(WIN_SIZE, dtype=np.float32) - HALF
    g = np.exp(-(coords ** 2) / np.float32(2.0 * SIGMA ** 2)).astype(np.float32)
    g = g / g.sum(dtype=np.float32)
    return g.astype(np.float32)


def _band_matrix_np():
    """[128, 3*542] bf16: pos | neg | half banded segments, 4 k-tiles each."""
    g = _gauss1d()
    A = np.zeros((H, H), dtype=np.float32)
    for i in range(H):
        lo = max(0, i - HALF)
        hi = min(H, i + HALF + 1)
        for j in range(lo, hi):
            A[i, j] = g[j - i + HALF]
    segs = []
    for kt in range(4):
        ns, w = WIN[kt]
        # R_kt[k', n] = A[n, kt*128+k']  -> shape [128, w]
        segs.append(A[ns:ns + w, kt * 128:(kt + 1) * 128].T.copy())
    cat = np.concatenate(segs, axis=1)
    assert cat.shape == (128, CATW)
    full = np.concatenate([cat, -cat, 0.5 * cat], axis=1)
    return full.astype(ml_dtypes.bfloat16)


def build_nc(planes=PLANES, prep="pool", dma="sync"):
    nc = bacc.Bacc(None)
    dmae = {"gpsimd": nc.gpsimd, "sync": nc.sync}[dma]
    prepe = {"pool": nc.gpsimd, "dve": nc.vector}[prep]
    pred_d = nc.declare_dram_parameter("pred", [planes, H, W], F32, isOutput=False)
    targ_d = nc.declare_dram_parameter("target", [planes, H, W], F32, isOutput=False)
    band_d = nc.declare_dram_parameter("bandmat", [128, BANDW], BF16, isOutput=False)
    acc_d = nc.declare_dram_parameter("acc", [128, planes], F32, isOutput=True)

    with tile.TileContext(nc) as tc:
        with (
            tc.tile_pool(name="const", bufs=1) as constp,
            tc.tile_pool(name="xy", bufs=2) as xyp,
            tc.tile_pool(name="fields", bufs=2) as fldp,
            tc.tile_pool(name="transposed", bufs=2) as trp,
            tc.tile_pool(name="post", bufs=1) as pp,
            tc.tile_pool(name="accp", bufs=1) as accp,
            tc.tile_pool(name="ps1", bufs=2, space="PSUM") as ps1,
            tc.tile_pool(name="ps2", bufs=1, space="PSUM") as ps2,
        ):
            BM = constp.tile([128, BANDW], BF16)
            dmae.dma_start(BM[:], band_d[:])
            acc = accp.tile([128, planes], F32)

            def band(var, kt):
                ns, w = WIN[kt]
                off = var * CATW + OFF[kt]
                return ns, w, off

            def conv_matmuls(dst_psum, srcs, blk, base):
                """dst_psum[p, n - base] += 1-D conv along the partition axis
                of each (src_tile, band_variant) in srcs, for the 128-col
                block `blk`. Output window cols are offset by -base."""
                mms = []
                for si, (T, var) in enumerate(srcs):
                    for kt in range(4):
                        lhsT = T[:, kt * 512 + blk * 128: kt * 512 + (blk + 1) * 128]
                        ns, w, off = band(var, kt)
                        if si == 0 and kt > 0:
                            mms.append((dst_psum[:, ns - base:ns - base + 10],
                                        lhsT, BM[:, off:off + 10]))
                            mms.append((dst_psum[:, ns - base + 10:ns - base + w],
                                        lhsT, BM[:, off + 10:off + w]))
                        else:
                            mms.append((dst_psum[:, ns - base:ns - base + w],
                                        lhsT, BM[:, off:off + w]))
                n_mm = len(mms)
                for i, (o, l, r) in enumerate(mms):
                    nc.tensor.matmul(o, l, r, start=(i == 0), stop=(i == n_mm - 1))

            for p in range(planes):
                X = xyp.tile([128, 2048], F32, tag="X")
                Y = xyp.tile([128, 2048], F32, tag="Y")
                dmae.dma_start(
                    X[:].rearrange("q (kt c) -> q kt c", kt=4),
                    pred_d[p].rearrange("(kt q) c -> q kt c", q=128))
                dmae.dma_start(
                    Y[:].rearrange("q (kt c) -> q kt c", kt=4),
                    targ_d[p].rearrange("(kt q) c -> q kt c", q=128))

                F1 = fldp.tile([128, 2048], BF16, tag="F1")
                F2 = fldp.tile([128, 2048], BF16, tag="F2")
                uF = fldp.tile([128, 2048], BF16, tag="uF")
                vF = fldp.tile([128, 2048], BF16, tag="vF")
                prepe.tensor_tensor(F1[:], X[:], Y[:], AOP.add)
                prepe.tensor_tensor(F2[:], X[:], Y[:], AOP.subtract)
                # bf16 in/out -> DVE 2x mode
                nc.vector.tensor_tensor(uF[:], F1[:], F1[:], AOP.mult)
                nc.vector.tensor_tensor(vF[:], F2[:], F2[:], AOP.mult)

                # pass 1: vertical blur + transpose, [128,1024] 2-bank psums,
                # extracted to bf16 T fields by ScalarE
                Ts = {}
                for nmf, ft, var in (("F1", F1, VPOS), ("F2", F2, VPOS),
                                     ("uF", uF, VHALF), ("vF", vF, VHALF)):
                    T = trp.tile([128, 2048], BF16, tag="T" + nmf)
                    for half in range(2):
                        ps = ps1.tile([128, 1024], F32, tag="p1")
                        for sub in range(2):
                            blk = half * 2 + sub
                            conv_matmuls(ps[:, sub * 512:(sub + 1) * 512],
                                         [(ft, var)], blk, base=0)
                        nc.scalar.copy(T[:, half * 1024:(half + 1) * 1024], ps[:])
                    Ts[nmf] = T

                # pass 2 per output-row block rc: 4 blurs, then post algebra
                u16 = pp.tile([128, 2048], BF16, tag="u16")
                v16 = pp.tile([128, 2048], BF16, tag="v16")
                up = pp.tile([128, 2048], BF16, tag="up")
                A1 = pp.tile([128, 2048], BF16, tag="A1")
                B1 = pp.tile([128, 2048], BF16, tag="B1")
                A2 = pp.tile([128, 2048], BF16, tag="A2")
                B2 = pp.tile([128, 2048], BF16, tag="B2")
                Nt = pp.tile([128, 2048], BF16, tag="Nt")
                Dt = pp.tile([128, 2048], BF16, tag="Dt")
                Rt = pp.tile([128, 2048], F32, tag="Rt")
                ssim = pp.tile([128, 2048], BF16, tag="ssim")

                for rc in range(4):
                    sl = slice(rc * 512, (rc + 1) * 512)
                    pss = ps2.tile([128, 512], F32, tag="pss")
                    psd = ps2.tile([128, 512], F32, tag="psd")
                    psU = ps2.tile([128, 512], F32, tag="psU")
                    psW = ps2.tile([128, 512], F32, tag="psW")
                    # pass-2 convs: dst windows are the full 512 cols of rc
                    conv_matmuls(pss, [(Ts["F1"], VPOS)], rc, base=0)
                    conv_matmuls(psd, [(Ts["F2"], VPOS)], rc, base=0)
                    conv_matmuls(psU, [(Ts["uF"], VPOS), (Ts["vF"], VPOS)],
                                 rc, base=0)
                    conv_matmuls(psW, [(Ts["uF"], VPOS), (Ts["vF"], VNEG)],
                                 rc, base=0)

                    # extraction + per-rc algebra
                    nc.scalar.activation(u16[:, sl], pss[:], AFT.Square,
                                         scale=INVR2)
                    nc.scalar.activation(v16[:, sl], psd[:], AFT.Square,
                                         scale=INVR2)
                    nc.vector.tensor_scalar_add(up[:, sl], u16[:, sl], C1)
                    nc.vector.tensor_tensor(A1[:, sl], up[:, sl], v16[:, sl],
                                            AOP.subtract)
                    nc.vector.tensor_tensor(B1[:, sl], up[:, sl], v16[:, sl],
                                            AOP.add)
                    nc.vector.scalar_tensor_tensor(A2[:, sl], psW[:], CC,
                                                   A1[:, sl], AOP.add,
                                                   AOP.subtract)
                    nc.vector.scalar_tensor_tensor(B2[:, sl], psU[:], CC,
                                                   B1[:, sl], AOP.add,
                                                   AOP.subtract)

                # plane-granularity finish
                nc.vector.tensor_tensor(Nt[:], A1[:], A2[:], AOP.mult)
                nc.vector.tensor_tensor(Dt[:], B1[:], B2[:], AOP.mult)
                nc.vector.reciprocal(Rt[:], Dt[:])
                # tensor_tensor_reduce hits a runtime INTERNAL error under
                # this PJRT path; scalar_tensor_tensor+accum_out is the same
                # fused multiply+row-sum in one DVE pass.
                nc.vector.scalar_tensor_tensor(
                    ssim[:], Nt[:], 1.0, Rt[:], AOP.mult, AOP.mult,
                    accum_out=acc[:, p: p + 1])

            dmae.dma_start(acc_d[:], acc[:])
    nc.compile()
    return nc


_CACHE = {}


def _get_nc():
    if "nc" not in _CACHE:
        _CACHE["nc"] = build_nc()
        _CACHE["band"] = _band_matrix_np()
    return _CACHE["nc"], _CACHE["band"]


def kernel(pred, target, _trace=False):
    pred = np.ascontiguousarray(np.asarray(pred), dtype=np.float32)
    target = np.ascontiguousarray(np.asarray(target), dtype=np.float32)
    nc, band = _get_nc()
    per = BATCH // N_CORES
    in_maps = []
    for i in range(N_CORES):
        in_maps.append({
            "pred": np.ascontiguousarray(
                pred[per * i: per * (i + 1)].reshape(PLANES, H, W)),
            "target": np.ascontiguousarray(
                target[per * i: per * (i + 1)].reshape(PLANES, H, W)),
            "bandmat": band,
        })
    kw = {}
    if _trace:
        kw["trace"] = True
    res = run_bass_kernel_spmd(nc, in_maps, list(range(N_CORES)), **kw)
    total = 0.0
    for r in res.results:
        total += float(np.asarray(r["acc"]).astype(np.float64).sum())
    loss = 1.0 - total / float(BATCH * CH * H * W)
    out = np.float32(loss)
    if _trace:
        return out, res
    return out


# revision 8
# speedup vs baseline: 2.4533x; 1.2249x over previous
"""SSIM loss kernel for Trainium2 (Bass/Tile), 8-core data parallel. v2: bf16.

Math (per 512x512 plane, 11x11 gaussian window G, zero "same" padding):
  mu_x = G*X, mu_y = G*Y
  ssim = ((2 mu_x mu_y + C1)(2 sg_xy + C2)) / ((mu_x^2+mu_y^2+C1)(sg_x+sg_y+C2))
  loss = 1 - mean(ssim)

Reformulation (s/d trick):
  F1 = X+Y, F2 = X-Y, uF = F1^2, vF = F2^2 (unscaled; the 1/2 is folded
  into the half-scaled band segment used for their pass-1 blur)
  s = G2(F1), d = G2(F2)        (G2 = 2-D blur, two banded-matmul passes)
  u = (s/sqrt2)^2, v = (d/sqrt2)^2
  psU = G2(uF)/2 + G2(vF)/2 = G2(X^2+Y^2)
  psW = G2(uF)/2 - G2(vF)/2 = 2 G2(XY)   (negated band for the vF stream)
  A1 = (u + C1) - v, B1 = (u + C1) + v
  A2 = (psW + CC) - A1, B2 = (psU + CC) - B1      (CC = C1+C2)
  ssim = (A1*A2) / (B1*B2)

All matmuls run in bf16 (1 col/cycle vs 4 for fp32). The blur is a banded
matmul with the image block stationary: matmul(out, lhsT=img_block,
rhs=band_cols) computes a 1-D conv along the partition axis plus a free
transpose; two passes give the separable 2-D blur back in natural layout.

Engine split per plane: GpSimd does F1/F2, ScalarE does squares + PSUM
extraction copies, VectorE does the bf16 algebra + reciprocal + reduce,
Sync triggers DMA. Host sums the per-partition partials in float64.
"""

import sys

for _p in ("/opt/trn_rl_repo",):
    if _p not in sys.path:
        sys.path.insert(0, _p)

import numpy as np
import ml_dtypes

import concourse.bass as bass
import concourse.bacc as bacc
import concourse.mybir as mybir
import concourse.tile as tile
from concourse.bass_utils import run_bass_kernel_spmd

F32 = mybir.dt.float32
LP = mybir.dt.float16  # fp16: same PE/DVE rates as bf16, 8x finer mantissa
AOP = mybir.AluOpType
AFT = mybir.ActivationFunctionType

N_CORES = 8
BATCH = 16
CH = 3
H = W = 512
PLANES = (BATCH // N_CORES) * CH  # 6 planes per core
WIN_SIZE = 11
SIGMA = 1.5
HALF = WIN_SIZE // 2
C1 = 0.01 ** 2
C2 = 0.03 ** 2
CC = C1 + C2
INVR2 = float(np.float32(1.0) / np.sqrt(np.float32(2.0)))

# per k-tile output-row windows [nstart, nstart+width) and offsets into one
# 542-wide band segment
WIN = [(0, 133), (123, 138), (251, 138), (379, 133)]
OFF = [0, 133, 271, 409]
CATW = 542  # 133+138+138+133
# band variants laid out side by side: positive, negated, half-scaled
VPOS, VNEG, VHALF = 0, 1, 2
BANDW = 3 * CATW


def _gauss1d():
    coords = np.arange(WIN_SIZE, dtype=np.float32) - HALF
    g = np.exp(-(coords ** 2) / np.float32(2.0 * SIGMA ** 2)).astype(np.float32)
    g = g / g.sum(dtype=np.float32)
    return g.astype(np.float32)


def _band_matrix_np():
    """[128, 3*542] bf16: pos | neg | half banded segments, 4 k-tiles each."""
    g = _gauss1d()
    A = np.zeros((H, H), dtype=np.float32)
    for i in range(H):
        lo = max(0, i - HALF)
        hi = min(H, i + HALF + 1)
        for j in range(lo, hi):
            A[i, j] = g[j - i + HALF]
    segs = []
    for kt in range(4):
        ns, w = WIN[kt]
        # R_kt[k', n] = A[n, kt*128+k']  -> shape [128, w]
        segs.append(A[ns:ns + w, kt * 128:(kt + 1) * 128].T.copy())
    cat = np.concatenate(segs, axis=1)
    assert cat.shape == (128, CATW)
    full = np.concatenate([cat, -cat, 0.5 * cat], axis=1)
    return full.astype(np.float16)


def build_nc(planes=PLANES, prep="pool", dma="sync"):
    nc = bacc.Bacc(None)
    dmae = {"gpsimd": nc.gpsimd, "sync": nc.sync}[dma]
    prepe = {"pool": nc.gpsimd, "dve": nc.vector}[prep]
    pred_d = nc.declare_dram_parameter("pred", [planes, H, W], F32, isOutput=False)
    targ_d = nc.declare_dram_parameter("target", [planes, H, W], F32, isOutput=False)
    band_d = nc.declare_dram_parameter("bandmat", [128, BANDW], LP, isOutput=False)
    acc_d = nc.declare_dram_parameter("acc", [128, planes], F32, isOutput=True)

    with tile.TileContext(nc) as tc:
        with (
            tc.tile_pool(name="const", bufs=1) as constp,
            tc.tile_pool(name="xy", bufs=2) as xyp,
            tc.tile_pool(name="fields", bufs=2) as fldp,
            tc.tile_pool(name="transposed", bufs=2) as trp,
            tc.tile_pool(name="post", bufs=1) as pp,
            tc.tile_pool(name="accp", bufs=1) as accp,
            tc.tile_pool(name="ps1", bufs=2, space="PSUM") as ps1,
            tc.tile_pool(name="ps2", bufs=1, space="PSUM") as ps2,
        ):
            BM = constp.tile([128, BANDW], LP)
            dmae.dma_start(BM[:], band_d[:])
            acc = accp.tile([128, planes], F32)

            def band(var, kt):
                ns, w = WIN[kt]
                off = var * CATW + OFF[kt]
                return ns, w, off

            def conv_matmuls(dst_psum, srcs, blk, base):
                """dst_psum[p, n - base] += 1-D conv along the partition axis
                of each (src_tile, band_variant) in srcs, for the 128-col
                block `blk`. Output window cols are offset by -base."""
                mms = []
                for si, (T, var) in enumerate(srcs):
                    for kt in range(4):
                        lhsT = T[:, kt * 512 + blk * 128: kt * 512 + (blk + 1) * 128]
                        ns, w, off = band(var, kt)
                        if si == 0 and kt > 0:
                            mms.append((dst_psum[:, ns - base:ns - base + 10],
                                        lhsT, BM[:, off:off + 10]))
                            mms.append((dst_psum[:, ns - base + 10:ns - base + w],
                                        lhsT, BM[:, off + 10:off + w]))
                        else:
                            mms.append((dst_psum[:, ns - base:ns - base + w],
                                        lhsT, BM[:, off:off + w]))
                n_mm = len(mms)
                for i, (o, l, r) in enumerate(mms):
                    nc.tensor.matmul(o, l, r, start=(i == 0), stop=(i == n_mm - 1))

            for p in range(planes):
                X = xyp.tile([128, 2048], F32, tag="X")
                Y = xyp.tile([128, 2048], F32, tag="Y")
                dmae.dma_start(
                    X[:].rearrange("q (kt c) -> q kt c", kt=4),
                    pred_d[p].rearrange("(kt q) c -> q kt c", q=128))
                dmae.dma_start(
                    Y[:].rearrange("q (kt c) -> q kt c", kt=4),
                    targ_d[p].rearrange("(kt q) c -> q kt c", q=128))

                F1 = fldp.tile([128, 2048], LP, tag="F1")
                F2 = fldp.tile([128, 2048], LP, tag="F2")
                uF = fldp.tile([128, 2048], LP, tag="uF")
                vF = fldp.tile([128, 2048], LP, tag="vF")
                prepe.tensor_tensor(F1[:], X[:], Y[:], AOP.add)
                prepe.tensor_tensor(F2[:], X[:], Y[:], AOP.subtract)
                # bf16 in/out -> DVE 2x mode
                nc.vector.tensor_tensor(uF[:], F1[:], F1[:], AOP.mult)
                nc.vector.tensor_tensor(vF[:], F2[:], F2[:], AOP.mult)

                # pass 1: vertical blur + transpose, [128,1024] 2-bank psums,
                # extracted to bf16 T fields by ScalarE
                Ts = {}
                for nmf, ft, var in (("F1", F1, VPOS), ("F2", F2, VPOS),
                                     ("uF", uF, VHALF), ("vF", vF, VHALF)):
                    T = trp.tile([128, 2048], LP, tag="T" + nmf)
                    for half in range(2):
                        ps = ps1.tile([128, 1024], F32, tag="p1")
                        for sub in range(2):
                            blk = half * 2 + sub
                            conv_matmuls(ps[:, sub * 512:(sub + 1) * 512],
                                         [(ft, var)], blk, base=0)
                        nc.scalar.copy(T[:, half * 1024:(half + 1) * 1024], ps[:])
                    Ts[nmf] = T

                # pass 2 per output-row block rc: 4 blurs, then post algebra
                u16 = pp.tile([128, 2048], LP, tag="u16")
                v16 = pp.tile([128, 2048], LP, tag="v16")
                up = pp.tile([128, 2048], LP, tag="up")
                A1 = pp.tile([128, 2048], LP, tag="A1")
                B1 = pp.tile([128, 2048], LP, tag="B1")
                A2 = pp.tile([128, 2048], LP, tag="A2")
                B2 = pp.tile([128, 2048], LP, tag="B2")
                Nt = pp.tile([128, 2048], LP, tag="Nt")
                Dt = pp.tile([128, 2048], F32, tag="Dt")
                Rt = pp.tile([128, 2048], F32, tag="Rt")
                ssim = pp.tile([128, 2048], LP, tag="ssim")

                for rc in range(4):
                    sl = slice(rc * 512, (rc + 1) * 512)
                    pss = ps2.tile([128, 512], F32, tag="pss")
                    psd = ps2.tile([128, 512], F32, tag="psd")
                    psU = ps2.tile([128, 512], F32, tag="psU")
                    psW = ps2.tile([128, 512], F32, tag="psW")
                    # pass-2 convs: dst windows are the full 512 cols of rc
                    conv_matmuls(pss, [(Ts["F1"], VPOS)], rc, base=0)
                    conv_matmuls(psd, [(Ts["F2"], VPOS)], rc, base=0)
                    conv_matmuls(psU, [(Ts["uF"], VPOS), (Ts["vF"], VPOS)],
                                 rc, base=0)
                    conv_matmuls(psW, [(Ts["uF"], VPOS), (Ts["vF"], VNEG)],
                                 rc, base=0)

                    # extraction + per-rc algebra
                    nc.scalar.activation(u16[:, sl], pss[:], AFT.Square,
                                         scale=INVR2)
                    nc.scalar.activation(v16[:, sl], psd[:], AFT.Square,
                                         scale=INVR2)
                    nc.vector.tensor_scalar_add(up[:, sl], u16[:, sl], C1)
                    nc.vector.tensor_tensor(A1[:, sl], up[:, sl], v16[:, sl],
                                            AOP.subtract)
                    nc.vector.tensor_tensor(B1[:, sl], up[:, sl], v16[:, sl],
                                            AOP.add)
                    nc.vector.scalar_tensor_tensor(A2[:, sl], psW[:], CC,
                                                   A1[:, sl], AOP.add,
                                                   AOP.subtract)
                    nc.vector.scalar_tensor_tensor(B2[:, sl], psU[:], CC,
                                                   B1[:, sl], AOP.add,
                                                   AOP.subtract)

                # plane-granularity finish
                nc.vector.tensor_tensor(Nt[:], A1[:], A2[:], AOP.mult)
                nc.vector.tensor_tensor(Dt[:], B1[:], B2[:], AOP.mult)
                scr = pp.tile([128, 2048], F32, tag="scr")
                nc.vector.reciprocal_approx_fast(Rt[:], Dt[:])
                del scr
                # tensor_tensor_reduce hits a runtime INTERNAL error under
                # this PJRT path; scalar_tensor_tensor+accum_out is the same
                # fused multiply+row-sum in one DVE pass.
                nc.vector.scalar_tensor_tensor(
                    ssim[:], Nt[:], 1.0, Rt[:], AOP.mult, AOP.mult,
                    accum_out=acc[:, p: p + 1])

            dmae.dma_start(acc_d[:], acc[:])
    nc.compile()
    return nc


_CACHE = {}


def _get_nc():
    if "nc" not in _CACHE:
        _CACHE["nc"] = build_nc()
        _CACHE["band"] = _band_matrix_np()
    return _CACHE["nc"], _CACHE["band"]


def kernel(pred, target, _trace=False):
    pred = np.ascontiguousarray(np.asarray(pred), dtype=np.float32)
    target = np.ascontiguousarray(np.asarray(target), dtype=np.float32)
    nc, band = _get_nc()
    per = BATCH // N_CORES
    in_maps = []
    for i in range(N_CORES):
        in_maps.append({
            "pred": np.ascontiguousarray(
                pred[per * i: per * (i + 1)].reshape(PLANES, H, W)),
            "target": np.ascontiguousarray(
                target[per * i: per * (i + 1)].reshape(PLANES, H, W)),
            "bandmat": band,
        })
    kw = {}
    if _trace:
        kw["trace"] = True
    res = run_bass_kernel_spmd(nc, in_maps, list(range(N_CORES)), **kw)
    total = 0.0
    for r in res.results:
        total += float(np.asarray(r["acc"]).astype(np.float64).sum())
    loss = 1.0 - total / float(BATCH * CH * H * W)
    out = np.float32(loss)
    if _trace:
        return out, res
    return out


# revision 12
# speedup vs baseline: 2.4778x; 1.0100x over previous
"""SSIM loss kernel for Trainium2 (Bass/Tile), 8-core data parallel. v2: bf16.

Math (per 512x512 plane, 11x11 gaussian window G, zero "same" padding):
  mu_x = G*X, mu_y = G*Y
  ssim = ((2 mu_x mu_y + C1)(2 sg_xy + C2)) / ((mu_x^2+mu_y^2+C1)(sg_x+sg_y+C2))
  loss = 1 - mean(ssim)

Reformulation (s/d trick):
  F1 = X+Y, F2 = X-Y, uF = F1^2, vF = F2^2 (unscaled; the 1/2 is folded
  into the half-scaled band segment used for their pass-1 blur)
  s = G2(F1), d = G2(F2)        (G2 = 2-D blur, two banded-matmul passes)
  u = (s/sqrt2)^2, v = (d/sqrt2)^2
  psU = G2(uF)/2 + G2(vF)/2 = G2(X^2+Y^2)
  psW = G2(uF)/2 - G2(vF)/2 = 2 G2(XY)   (negated band for the vF stream)
  A1 = (u + C1) - v, B1 = (u + C1) + v
  A2 = (psW + CC) - A1, B2 = (psU + CC) - B1      (CC = C1+C2)
  ssim = (A1*A2) / (B1*B2)

All matmuls run in bf16 (1 col/cycle vs 4 for fp32). The blur is a banded
matmul with the image block stationary: matmul(out, lhsT=img_block,
rhs=band_cols) computes a 1-D conv along the partition axis plus a free
transpose; two passes give the separable 2-D blur back in natural layout.

Engine split per plane: GpSimd does F1/F2, ScalarE does squares + PSUM
extraction copies, VectorE does the bf16 algebra + reciprocal + reduce,
Sync triggers DMA. Host sums the per-partition partials in float64.
"""

import sys

for _p in ("/opt/trn_rl_repo",):
    if _p not in sys.path:
        sys.path.insert(0, _p)

import numpy as np
import ml_dtypes

import concourse.bass as bass
import concourse.bacc as bacc
import concourse.mybir as mybir
import concourse.tile as tile
from concourse.bass_utils import run_bass_kernel_spmd

F32 = mybir.dt.float32
LP = mybir.dt.float16  # fp16: same PE/DVE rates as bf16, 8x finer mantissa
AOP = mybir.AluOpType
AFT = mybir.ActivationFunctionType

N_CORES = 8
BATCH = 16
CH = 3
H = W = 512
PLANES = (BATCH // N_CORES) * CH  # 6 planes per core
WIN_SIZE = 11
SIGMA = 1.5
HALF = WIN_SIZE // 2
C1 = 0.01 ** 2
C2 = 0.03 ** 2
CC = C1 + C2
INVR2 = float(np.float32(1.0) / np.sqrt(np.float32(2.0)))

# per k-tile output-row windows [nstart, nstart+width) and offsets into one
# 542-wide band segment
WIN = [(0, 133), (123, 138), (251, 138), (379, 133)]
OFF = [0, 133, 271, 409]
CATW = 542  # 133+138+138+133
# band variants laid out side by side: positive, negated, half-scaled
VPOS, VNEG, VHALF = 0, 1, 2
BANDW = 3 * CATW


def _gauss1d():
    coords = np.arange(WIN_SIZE, dtype=np.float32) - HALF
    g = np.exp(-(coords ** 2) / np.float32(2.0 * SIGMA ** 2)).astype(np.float32)
    g = g / g.sum(dtype=np.float32)
    return g.astype(np.float32)


def _band_matrix_np():
    """[128, 3*542] bf16: pos | neg | half banded segments, 4 k-tiles each."""
    g = _gauss1d()
    A = np.zeros((H, H), dtype=np.float32)
    for i in range(H):
        lo = max(0, i - HALF)
        hi = min(H, i + HALF + 1)
        for j in range(lo, hi):
            A[i, j] = g[j - i + HALF]
    segs = []
    for kt in range(4):
        ns, w = WIN[kt]
        # R_kt[k', n] = A[n, kt*128+k']  -> shape [128, w]
        segs.append(A[ns:ns + w, kt * 128:(kt + 1) * 128].T.copy())
    cat = np.concatenate(segs, axis=1)
    assert cat.shape == (128, CATW)
    full = np.concatenate([cat, -cat, 0.5 * cat], axis=1)
    return full.astype(np.float16)


def build_nc(planes=PLANES, prep="pool", dma="sync"):
    nc = bacc.Bacc(None)
    dmae = {"gpsimd": nc.gpsimd, "sync": nc.sync}[dma]
    prepe = {"pool": nc.gpsimd, "dve": nc.vector}[prep]
    pred_d = nc.declare_dram_parameter("pred", [planes, H, W], F32, isOutput=False)
    targ_d = nc.declare_dram_parameter("target", [planes, H, W], F32, isOutput=False)
    band_d = nc.declare_dram_parameter("bandmat", [128, BANDW], LP, isOutput=False)
    acc_d = nc.declare_dram_parameter("acc", [128, planes], F32, isOutput=True)

    with tile.TileContext(nc) as tc:
        with (
            tc.tile_pool(name="const", bufs=1) as constp,
            tc.tile_pool(name="xy", bufs=2) as xyp,
            tc.tile_pool(name="fields", bufs=2) as fldp,
            tc.tile_pool(name="transposed", bufs=2) as trp,
            tc.tile_pool(name="post", bufs=1) as pp,
            tc.tile_pool(name="accp", bufs=1) as accp,
            tc.tile_pool(name="ps1", bufs=2, space="PSUM") as ps1,
            tc.tile_pool(name="ps2", bufs=1, space="PSUM") as ps2,
        ):
            BM = constp.tile([128, BANDW], LP)
            dmae.dma_start(BM[:], band_d[:])
            acc = accp.tile([128, planes], F32)

            def band(var, kt):
                ns, w = WIN[kt]
                off = var * CATW + OFF[kt]
                return ns, w, off

            def conv_matmuls(dst_psum, srcs, blk, base):
                """dst_psum[p, n - base] += 1-D conv along the partition axis
                of each (src_tile, band_variant) in srcs, for the 128-col
                block `blk`. Output window cols are offset by -base."""
                mms = []
                for si, (T, var) in enumerate(srcs):
                    for kt in range(4):
                        lhsT = T[:, kt * 512 + blk * 128: kt * 512 + (blk + 1) * 128]
                        ns, w, off = band(var, kt)
                        # overlapping output windows accumulate correctly:
                        # start=True clears has_written for the whole bank
                        mms.append((dst_psum[:, ns - base:ns - base + w],
                                    lhsT, BM[:, off:off + w]))
                n_mm = len(mms)
                for i, (o, l, r) in enumerate(mms):
                    nc.tensor.matmul(o, l, r, start=(i == 0), stop=(i == n_mm - 1))

            for p in range(planes):
                X = xyp.tile([128, 2048], F32, tag="X")
                Y = xyp.tile([128, 2048], F32, tag="Y")
                dmae.dma_start(
                    X[:].rearrange("q (kt c) -> q kt c", kt=4),
                    pred_d[p].rearrange("(kt q) c -> q kt c", q=128))
                dmae.dma_start(
                    Y[:].rearrange("q (kt c) -> q kt c", kt=4),
                    targ_d[p].rearrange("(kt q) c -> q kt c", q=128))

                F1 = fldp.tile([128, 2048], LP, tag="F1")
                F2 = fldp.tile([128, 2048], LP, tag="F2")
                uF = fldp.tile([128, 2048], LP, tag="uF")
                vF = fldp.tile([128, 2048], LP, tag="vF")
                prepe.tensor_tensor(F1[:], X[:], Y[:], AOP.add)
                prepe.tensor_tensor(F2[:], X[:], Y[:], AOP.subtract)
                # split the squares across ACT and DVE (engine balance)
                nc.scalar.activation(uF[:], F1[:], AFT.Square)
                nc.vector.tensor_tensor(vF[:], F2[:], F2[:], AOP.mult)

                # pass 1: vertical blur + transpose, [128,1024] 2-bank psums,
                # extracted to bf16 T fields by ScalarE
                Ts = {}
                for nmf, ft, var in (("F1", F1, VPOS), ("F2", F2, VPOS),
                                     ("uF", uF, VHALF), ("vF", vF, VHALF)):
                    T = trp.tile([128, 2048], LP, tag="T" + nmf)
                    for half in range(2):
                        ps = ps1.tile([128, 1024], F32, tag="p1")
                        for sub in range(2):
                            blk = half * 2 + sub
                            conv_matmuls(ps[:, sub * 512:(sub + 1) * 512],
                                         [(ft, var)], blk, base=0)
                        nc.scalar.copy(T[:, half * 1024:(half + 1) * 1024], ps[:])
                    Ts[nmf] = T

                # pass 2 per output-row block rc: 4 blurs, then post algebra
                u16 = pp.tile([128, 2048], LP, tag="u16")
                v16 = pp.tile([128, 2048], LP, tag="v16")
                up = pp.tile([128, 2048], LP, tag="up")
                A1 = pp.tile([128, 2048], LP, tag="A1")
                B1 = pp.tile([128, 2048], LP, tag="B1")
                A2 = pp.tile([128, 2048], LP, tag="A2")
                B2 = pp.tile([128, 2048], LP, tag="B2")
                Nt = pp.tile([128, 2048], LP, tag="Nt")
                Dt = pp.tile([128, 2048], F32, tag="Dt")
                Rt = pp.tile([128, 2048], F32, tag="Rt")
                ssim = pp.tile([128, 2048], LP, tag="ssim")

                for rc in range(4):
                    sl = slice(rc * 512, (rc + 1) * 512)
                    pss = ps2.tile([128, 512], F32, tag="pss")
                    psd = ps2.tile([128, 512], F32, tag="psd")
                    psU = ps2.tile([128, 512], F32, tag="psU")
                    psW = ps2.tile([128, 512], F32, tag="psW")
                    # pass-2 convs: dst windows are the full 512 cols of rc
                    conv_matmuls(pss, [(Ts["F1"], VPOS)], rc, base=0)
                    conv_matmuls(psd, [(Ts["F2"], VPOS)], rc, base=0)
                    conv_matmuls(psU, [(Ts["uF"], VPOS), (Ts["vF"], VPOS)],
                                 rc, base=0)
                    conv_matmuls(psW, [(Ts["uF"], VPOS), (Ts["vF"], VNEG)],
                                 rc, base=0)

                    # extraction + per-rc algebra
                    nc.scalar.activation(u16[:, sl], pss[:], AFT.Square,
                                         scale=INVR2)
                    nc.scalar.activation(v16[:, sl], psd[:], AFT.Square,
                                         scale=INVR2)
                    nc.vector.tensor_scalar_add(up[:, sl], u16[:, sl], C1)
                    nc.vector.tensor_tensor(A1[:, sl], up[:, sl], v16[:, sl],
                                            AOP.subtract)
                    nc.vector.tensor_tensor(B1[:, sl], up[:, sl], v16[:, sl],
                                            AOP.add)
                    nc.vector.scalar_tensor_tensor(A2[:, sl], psW[:], CC,
                                                   A1[:, sl], AOP.add,
                                                   AOP.subtract)
                    nc.vector.scalar_tensor_tensor(B2[:, sl], psU[:], CC,
                                                   B1[:, sl], AOP.add,
                                                   AOP.subtract)

                # plane-granularity finish
                nc.vector.tensor_tensor(Nt[:], A1[:], A2[:], AOP.mult)
                nc.gpsimd.tensor_tensor(Dt[:], B1[:], B2[:], AOP.mult)
                nc.vector.reciprocal_approx_fast(Rt[:], Dt[:])
                # tensor_tensor_reduce hits a runtime INTERNAL error under
                # this PJRT path; scalar_tensor_tensor+accum_out is the same
                # fused multiply+row-sum in one DVE pass.
                nc.vector.scalar_tensor_tensor(
                    ssim[:], Nt[:], 1.0, Rt[:], AOP.mult, AOP.mult,
                    accum_out=acc[:, p: p + 1])

            dmae.dma_start(acc_d[:], acc[:])
    nc.compile()
    return nc


_CACHE = {}


def _get_nc():
    if "nc" not in _CACHE:
        _CACHE["nc"] = build_nc()
        _CACHE["band"] = _band_matrix_np()
    return _CACHE["nc"], _CACHE["band"]


def kernel(pred, target, _trace=False):
    pred = np.ascontiguousarray(np.asarray(pred), dtype=np.float32)
    target = np.ascontiguousarray(np.asarray(target), dtype=np.float32)
    nc, band = _get_nc()
    per = BATCH // N_CORES
    in_maps = []
    for i in range(N_CORES):
        in_maps.append({
            "pred": np.ascontiguousarray(
                pred[per * i: per * (i + 1)].reshape(PLANES, H, W)),
            "target": np.ascontiguousarray(
                target[per * i: per * (i + 1)].reshape(PLANES, H, W)),
            "bandmat": band,
        })
    kw = {}
    if _trace:
        kw["trace"] = True
    res = run_bass_kernel_spmd(nc, in_maps, list(range(N_CORES)), **kw)
    total = 0.0
    for r in res.results:
        total += float(np.asarray(r["acc"]).astype(np.float64).sum())
    loss = 1.0 - total / float(BATCH * CH * H * W)
    out = np.float32(loss)
    if _trace:
        return out, res
    return out
